# revision 39
# baseline (speedup 1.0000x reference)
"""GCN layer kernel for Trainium2 (8 NeuronCores, SPMD).

out = segment_sum(norm * (x @ W)[col] by row), norm = deg^-1/2[row]*deg^-1/2[col],
with self-loops appended.

Strategy (memory-regime, SWDGE-descriptor-rate bound):
  - Reformulate: out[r] = dis[r] * (sum_{e: row=r} xs[col_e]) @ W with
    xs = dis[:,None]*x: gather raw xs rows (no x@W materialization), apply W
    once per 128-row output tile, scale by dis[row] at the end.
  - Shard output rows across 8 cores (12500 rows each). Per core:
      * dma_gather (SWDGE int16 idx, 2048 idx/call, queue = source bucket)
        pulls edge source rows (bf16) from HBM into SBUF call-tiles of up to
        16 chunks x 128 edges. Calls stream bucket-major ACROSS supertile
        boundaries so there are no small remainder calls.
      * per chunk, PE accumulates G^T @ S into a [128 feat x 512 slot] fp32
        PSUM bank; S is a one-hot [128 x 64] window GENERATED ON-CHIP by DVE
        (is_equal of a broadcast iota row against per-(lane,chunk) slot ids),
        so no S matrix is ever read from HBM.
      * the PSUM bank is zeroed once by DVE; self-loop contributions enter
        via PE transpose of the core's own fp32 rows; all matmuls accumulate
        (start=False) so the Tile scheduler may reorder them freely.
      * per supertile: ACT copies PSUM->SBUF, PE applies W (fp32), DVE scales
        by dis[row], DMA out.
  - Col-buckets of 25000 rows keep gather indices within int16 range.
  - One shared chunk schedule for all 8 cores (SPMD: one NEFF); per-core edge
    data is packed into the schedule with padding (idx=0 lanes get S=0 via
    slot id -1).
"""

import ml_dtypes
import numpy as np

import concourse.mybir as mybir
import concourse.tile as tile
from concourse import bacc
from concourse.bass_utils import run_bass_kernel_spmd
from concourse.masks import make_identity

N_NODES = 100000
N_EDGES = 1600000
D = 128
P = 128
NCORES = 8
RPC = N_NODES // NCORES            # rows per core = 12500
SLOTS = 512                        # slots per supertile (one PSUM bank, f32)
NST = (RPC + SLOTS - 1) // SLOTS   # 25 supertiles (last has 212 slots)
NBUCK = 4
BUCK = 25000                       # bucket size (int16-safe gather indices)
WWIN = 128                         # selection-matrix window width
GMAX = 8                           # max chunks per dma_gather call (1024 idx:
                                   # hard SWDGE ucode limit, 2048 crashes even
                                   # with a 32KB descriptor ring)
NQUEUES = 4                        # SWDGE queues; queue = bucket
GBUFS = 20                         # in-flight gather call tiles
F32 = mybir.dt.float32
BF16 = mybir.dt.bfloat16
I16 = mybir.dt.int16
BF = ml_dtypes.bfloat16

_compiled = {}


def _spread_bases(n_win, maxbase):
    """Evenly spread n_win window bases over [0, maxbase]."""
    if n_win <= 0:
        return []
    if n_win == 1:
        return [maxbase // 2 if maxbase > 0 else 0]
    return [i * maxbase // (n_win - 1) for i in range(n_win)]


def _chunk_layout(st, b, C):
    """Return (bases, widths) for C chunks of group (st, b)."""
    slots_st = min(SLOTS, RPC - st * SLOTS)
    maxbase = max(0, slots_st - WWIN)
    bases = _spread_bases(C, maxbase)
    widths = [WWIN] * C
    return bases, widths


def _assign(slots_arr, bases, widths):
    """Greedy interval assignment of edges (sorted by slot) to chunks.

    Returns list of per-chunk edge-index lists, or None if infeasible."""
    C = len(bases)
    E = len(slots_arr)
    cap = [[] for _ in range(C)]
    leftover = []
    ptr = 0
    for k in range(C):
        B = bases[k]
        end = B + widths[k]
        while ptr < E and slots_arr[ptr] < B:
            leftover.append(ptr)
            ptr += 1
        while ptr < E and slots_arr[ptr] < end and len(cap[k]) < P:
            cap[k].append(ptr)
            ptr += 1
    leftover.extend(range(ptr, E))
    for e in leftover:
        s = slots_arr[e]
        for k in range(C):
            if bases[k] <= s < bases[k] + widths[k] and len(cap[k]) < P:
                cap[k].append(e)
                break
        else:
            return None
    return cap


def _prepare(x, edge_index, W, nst_limit=NST):
    """Host-side preprocessing: degrees, xs tables, per-core packed metadata
    (gather indices + slot ids for on-chip S generation) and the shared
    bucket-major call schedule."""
    row = np.asarray(edge_index[0], dtype=np.int64)
    col = np.asarray(edge_index[1], dtype=np.int64)
    full_row = np.concatenate([row, np.arange(N_NODES, dtype=np.int64)])
    deg = np.bincount(full_row, minlength=N_NODES).astype(np.float64)
    dis = (1.0 / np.sqrt(deg)).astype(np.float32)
    xs32 = (x * dis[:, None]).astype(np.float32)
    xs16 = xs32.astype(BF)

    # only the original edges go through the gather path (self-loops are
    # handled by the transpose init)
    core = row // RPC
    lrow = (row - core * RPC).astype(np.int64)
    st_all = lrow // SLOTS
    slot_all = lrow % SLOTS
    buck_all = col // BUCK
    colrel_all = (col - buck_all * BUCK).astype(np.int64)

    order = np.lexsort((slot_all, buck_all, st_all, core))
    core_s = core[order]
    st_s = st_all[order]
    b_s = buck_all[order]
    slot_s = slot_all[order].astype(np.int64)
    colrel_s = colrel_all[order].astype(np.int64)

    key = ((core_s * NST) + st_s) * NBUCK + b_s
    bounds = np.searchsorted(key, np.arange(NCORES * NST * NBUCK + 1))

    def group(c, st, b):
        g = (c * NST + st) * NBUCK + b
        lo, hi = bounds[g], bounds[g + 1]
        return slot_s[lo:hi], colrel_s[lo:hi]

    # chunk counts: per-(st,b) max over cores of ceil(E/128), floored so the
    # evenly-spread windows cover every slot
    C = np.zeros((NST, NBUCK), dtype=np.int64)
    for st in range(nst_limit):
        slots_st = min(SLOTS, RPC - st * SLOTS)
        cover = max(1, -(-max(0, slots_st - WWIN) // WWIN) + 1)
        for b in range(NBUCK):
            mx = cover
            for c in range(NCORES):
                lo = bounds[(c * NST + st) * NBUCK + b]
                hi = bounds[(c * NST + st) * NBUCK + b + 1]
                mx = max(mx, -((lo - hi) // P))
            C[st, b] = mx

    assigns = {}
    for st in range(nst_limit):
        for b in range(NBUCK):
            while True:
                bases, widths = _chunk_layout(st, b, int(C[st, b]))
                ok = True
                for c in range(NCORES):
                    sl, _ = group(c, st, b)
                    a = _assign(sl, bases, widths)
                    if a is None:
                        ok = False
                        break
                    assigns[(c, st, b)] = a
                if ok:
                    break
                C[st, b] += 1
                if C[st, b] > 200:
                    raise RuntimeError(f"packing diverged at st={st} b={b}")

    # bucket-major global chunk order: (b, st, k); calls are consecutive runs
    # of up to GMAX chunks within one bucket (crossing supertile boundaries)
    schedule = []           # [st][b] -> (C, bases, widths)
    for st in range(nst_limit):
        per_b = []
        for b in range(NBUCK):
            bases, widths = _chunk_layout(st, b, int(C[st, b]))
            per_b.append((int(C[st, b]), bases, widths))
        schedule.append(per_b)

    gmap = {}               # (b, st, k) -> global chunk idx (bucket-major)
    gc = 0
    for b in range(NBUCK):
        for st in range(nst_limit):
            for k in range(int(C[st, b])):
                gmap[(b, st, k)] = gc
                gc += 1
    total_chunks = gc

    calls = []              # (b, g0, n): chunks g0..g0+n-1 (global idx)
    chunk_call = {}         # global chunk idx -> (call_idx, j)
    for b in range(NBUCK):
        nb = sum(int(C[st, b]) for st in range(nst_limit))
        g0 = gmap[(b, 0, 0)] if nb else 0
        off = 0
        while off < nb:
            n = min(GMAX, nb - off)
            for j in range(n):
                chunk_call[g0 + off + j] = (len(calls), j)
            calls.append((b, g0 + off, n))
            off += n

    idx_cols = total_chunks * (P // 16)

    slot_meta = np.full((NCORES, P, total_chunks), -1.0, dtype=BF)
    idx_meta = np.zeros((NCORES, P, idx_cols), dtype=np.int16)
    for c in range(NCORES):
        for st in range(nst_limit):
            for b in range(NBUCK):
                Cb, bases, widths = schedule[st][b]
                sl, cr = group(c, st, b)
                a = assigns[(c, st, b)]
                for k in range(Cb):
                    g = gmap[(b, st, k)]
                    edges = a[k]
                    ne = len(edges)
                    idx_flat = np.zeros(P, dtype=np.int16)
                    if ne:
                        e = np.asarray(edges, dtype=np.int64)
                        lanes = np.arange(ne)
                        slot_meta[c, lanes, g] = (sl[e] - bases[k]).astype(BF)
                        idx_flat[:ne] = cr[e].astype(np.int16)
                    wrapped = idx_flat.reshape(8, 16).T    # [16, 8]
                    idx_meta[c, :, g * 8:(g + 1) * 8] = np.tile(wrapped, (8, 1))

    ntiles = (RPC + P - 1) // P  # 98
    dis_meta = np.ones((NCORES, P, ntiles), dtype=np.float32)
    for c in range(NCORES):
        dd = dis[c * RPC:(c + 1) * RPC]
        pad = np.ones(ntiles * P, dtype=np.float32)
        pad[:RPC] = dd
        dis_meta[c] = pad.reshape(ntiles, P).T

    iota = np.tile(np.arange(WWIN, dtype=BF)[None, :], (P, 1))

    return (xs16, xs32, schedule, total_chunks, idx_cols, calls, chunk_call,
            gmap, slot_meta, idx_meta, dis_meta, iota)


def _build_program(schedule, total_chunks, idx_cols, calls, chunk_call, gmap):
    nst_limit = len(schedule)
    nc = bacc.Bacc("TRN2", target_bir_lowering=False, num_swdge_queues=NQUEUES)
    ntiles = (RPC + P - 1) // P

    xs_d = nc.dram_tensor("xs", [N_NODES, D], BF16, kind="ExternalInput")
    xself_d = nc.dram_tensor("xself", [RPC, D], BF16, kind="ExternalInput")
    idx_d = nc.dram_tensor("idx", [P, idx_cols], I16, kind="ExternalInput")
    slot_d = nc.dram_tensor("slot", [P, total_chunks], BF16,
                            kind="ExternalInput")
    iota_d = nc.dram_tensor("iota", [P, WWIN], BF16, kind="ExternalInput")
    w_d = nc.dram_tensor("w", [D, D], F32, kind="ExternalInput")
    dis_d = nc.dram_tensor("dis", [P, ntiles], F32, kind="ExternalInput")
    out_d = nc.dram_tensor("out", [RPC, D], F32, kind="ExternalOutput")

    with tile.TileContext(nc) as tc:
        with tc.tile_pool(name="const", bufs=1) as const, \
             tc.tile_pool(name="g", bufs=GBUFS) as gp, \
             tc.tile_pool(name="sg", bufs=GBUFS) as sgp, \
             tc.tile_pool(name="xl", bufs=3) as xlp, \
             tc.tile_pool(name="misc", bufs=3) as misc, \
             tc.tile_pool(name="pacc", bufs=3, space="PSUM") as pacc, \
             tc.tile_pool(name="pout", bufs=2, space="PSUM") as pout:

            # index tables: one tiny "head" tile per bucket holding just the
            # first call's indices (loaded first, on the scalar/ACT HWDGE
            # ring, so all 4 SWDGE contexts start grinding ASAP), then one
            # main tile per bucket loaded by a single DMA
            bstarts = [gmap[(b, 0, 0)] * 8 for b in range(NBUCK)] + [idx_cols]
            head_ts = []
            for b in range(NBUCK):
                lo = bstarts[b]
                t = const.tile([P, GMAX * 8], I16, tag=f"idxh{b}")
                nc.scalar.dma_start(t[:], idx_d[:, lo:lo + GMAX * 8])
                head_ts.append(t)
            slot_t = const.tile([P, total_chunks], BF16, tag="slot")
            nc.scalar.dma_start(slot_t[:], slot_d[:, :])
            iota_t = const.tile([P, WWIN], BF16, tag="iota")
            nc.scalar.dma_start(iota_t[:], iota_d[:, :])
            idx_ts = []
            for b in range(NBUCK):
                lo, hi = bstarts[b], bstarts[b + 1]
                t = const.tile([P, hi - lo], I16, tag=f"idx{b}")
                nc.scalar.dma_start(t[:], idx_d[:, lo:hi])
                idx_ts.append(t)
            w_t = const.tile([D, D], F32, tag="w")
            nc.sync.dma_start(w_t[:], w_d[:, :])
            dis_t = const.tile([P, ntiles], F32, tag="dis")
            nc.sync.dma_start(dis_t[:], dis_d[:, :])
            ident_t = const.tile([P, P], BF16, tag="ident")
            make_identity(nc, ident_t[:])

            call_tiles = {}
            emitted = set()
            qrot = [0]

            def emit_call(cid):
                b, g0, n = calls[cid]
                gt = gp.tile([P, GMAX, P], BF16, tag="g")
                nrows = min(BUCK, N_NODES - b * BUCK)
                lo = bstarts[b]
                if g0 * 8 == lo:
                    src = head_ts[b][:, :n * 8]
                else:
                    src = idx_ts[b][:, g0 * 8 - lo:(g0 + n) * 8 - lo]
                nc.gpsimd.dma_gather(
                    out_ap=gt[:, :n, :],
                    in_ap=xs_d[b * BUCK:b * BUCK + nrows, :],
                    idxs_ap=src,
                    num_idxs=n * P,
                    num_idxs_reg=n * P,
                    elem_size=D,
                    queue_num=qrot[0] % NQUEUES,
                )
                qrot[0] += 1
                sgt = sgp.tile([P, GMAX, WWIN], BF16, tag="sg")
                nc.vector.tensor_tensor(
                    out=sgt[:, :n, :],
                    in0=iota_t[:, :].unsqueeze(1).broadcast_to([P, n, WWIN]),
                    in1=slot_t[:, g0:g0 + n].unsqueeze(2)
                        .broadcast_to([P, n, WWIN]),
                    op=mybir.AluOpType.is_equal,
                )
                call_tiles[cid] = (gt, sgt)
                emitted.add(cid)

            for st in range(nst_limit):
                accT = pacc.tile([P, SLOTS], F32, tag="acc")
                rows_st = min(SLOTS, RPC - st * SLOTS)
                nsub = (rows_st + P - 1) // P

                # zero the PSUM bank once; every matmul below accumulates, so
                # the scheduler may order them freely
                nc.vector.memset(accT[:], 0.0)

                # self-loop contributions via PE transpose of own rows (bf16,
                # all full subtiles of the supertile batched into one DMA)
                xsel = xlp.tile([P, 4, D], BF16, tag="xl")
                nfull = rows_st // P
                if nfull:
                    r0 = st * SLOTS
                    nc.sync.dma_start(
                        xsel[:, :nfull, :],
                        xself_d[r0:r0 + nfull * P, :].rearrange(
                            "(s p) d -> p s d", p=P),
                    )
                for sub in range(nsub):
                    rows = min(P, rows_st - sub * P)
                    if rows < P:
                        xtail = xlp.tile([P, D], BF16, tag="xt")
                        nc.vector.memset(xtail[:], 0.0)
                        r0 = st * SLOTS + sub * P
                        nc.sync.dma_start(xtail[:rows, :],
                                          xself_d[r0:r0 + rows, :])
                        lhs = xtail[:]
                    else:
                        lhs = xsel[:, sub, :]
                    # regular accumulating matmul against the identity acts as
                    # a transpose (bf16 in, fp32 PSUM accumulate)
                    nc.tensor.matmul(
                        out=accT[:, sub * P:(sub + 1) * P],
                        lhsT=lhs,
                        rhs=ident_t[:],
                        start=False, stop=False,
                        skip_group_check=True,
                    )

                nchunks_st = sum(schedule[st][b][0] for b in range(NBUCK))
                done = 0
                for b in range(NBUCK):
                    Cb, bases, widths = schedule[st][b]
                    for k in range(Cb):
                        g = gmap[(b, st, k)]
                        cid, j = chunk_call[g]
                        if cid not in emitted:
                            emit_call(cid)
                        gt, sgt = call_tiles[cid]
                        done += 1
                        nc.tensor.matmul(
                            out=accT[:, bases[k]:bases[k] + widths[k]],
                            lhsT=gt[:, j, :],
                            rhs=sgt[:, j, :widths[k]],
                            start=False,
                            stop=(done == nchunks_st),
                            skip_group_check=True,
                        )

                accT_s = misc.tile([P, SLOTS], F32, tag="accs")
                nc.scalar.copy(out=accT_s[:], in_=accT[:])
                for sub in range(nsub):
                    rows = min(P, rows_st - sub * P)
                    op_t = pout.tile([P, D], F32, tag="op")
                    nc.tensor.matmul(
                        out=op_t[:],
                        lhsT=accT_s[:, sub * P:(sub + 1) * P],
                        rhs=w_t[:],
                        start=True, stop=True,
                    )
                    os_t = misc.tile([P, D], F32, tag="os")
                    nc.vector.tensor_scalar(
                        out=os_t[:],
                        in0=op_t[:],
                        scalar1=dis_t[:, st * 4 + sub:st * 4 + sub + 1],
                        scalar2=None,
                        op0=mybir.AluOpType.mult,
                    )
                    r0 = st * SLOTS + sub * P
                    nc.sync.dma_start(out_d[r0:r0 + rows, :], os_t[:rows, :])

    nc.compile()
    return nc


def kernel(x, edge_index, W, trace=False):
    import sys
    import time as _time
    x = np.ascontiguousarray(np.asarray(x, dtype=np.float32))
    edge_index = np.asarray(edge_index)
    W = np.ascontiguousarray(np.asarray(W, dtype=np.float32))

    t0 = _time.time()
    (xs16, xs32, schedule, total_chunks, idx_cols, calls, chunk_call, gmap,
     slot_meta, idx_meta, dis_meta, iota) = _prepare(x, edge_index, W)
    print(f"[kernel] prepare {_time.time()-t0:.1f}s, total_chunks={total_chunks}"
          f", calls={len(calls)}", file=sys.stderr)

    key = tuple(
        (schedule[st][b][0],) + tuple(schedule[st][b][1])
        for st in range(len(schedule)) for b in range(NBUCK)
    )
    if key not in _compiled:
        _compiled.clear()
        t0 = _time.time()
        _compiled[key] = _build_program(schedule, total_chunks, idx_cols,
                                        calls, chunk_call, gmap)
        print(f"[kernel] build+schedule {_time.time()-t0:.1f}s", file=sys.stderr)
    nc = _compiled[key]

    in_maps = []
    for c in range(NCORES):
        in_maps.append({
            "xs": xs16,
            "xself": np.ascontiguousarray(xs16[c * RPC:(c + 1) * RPC]),
            "idx": np.ascontiguousarray(idx_meta[c]),
            "slot": np.ascontiguousarray(slot_meta[c]),
            "iota": iota,
            "w": W,
            "dis": np.ascontiguousarray(dis_meta[c]),
        })

    res = run_bass_kernel_spmd(nc, in_maps, core_ids=list(range(NCORES)),
                               trace=trace)
    out = np.concatenate([res.results[c]["out"] for c in range(NCORES)], axis=0)
    kernel._last_results = res
    return out


# revision 42
# speedup vs baseline: 1.0218x; 1.0218x over previous
"""GCN layer kernel for Trainium2 (8 NeuronCores, SPMD).

out = segment_sum(norm * (x @ W)[col] by row), norm = deg^-1/2[row]*deg^-1/2[col],
with self-loops appended.

Strategy (memory-regime, SWDGE-descriptor-rate bound):
  - Reformulate: out[r] = dis[r] * (sum_{e: row=r} xs[col_e]) @ W with
    xs = dis[:,None]*x: gather raw xs rows (no x@W materialization), apply W
    once per 128-row output tile, scale by dis[row] at the end.
  - Shard output rows across 8 cores (12500 rows each). Per core:
      * dma_gather (SWDGE int16 idx, 2048 idx/call, queue = source bucket)
        pulls edge source rows (bf16) from HBM into SBUF call-tiles of up to
        16 chunks x 128 edges. Calls stream bucket-major ACROSS supertile
        boundaries so there are no small remainder calls.
      * per chunk, PE accumulates G^T @ S into a [128 feat x 512 slot] fp32
        PSUM bank; S is a one-hot [128 x 64] window GENERATED ON-CHIP by DVE
        (is_equal of a broadcast iota row against per-(lane,chunk) slot ids),
        so no S matrix is ever read from HBM.
      * the PSUM bank is zeroed once by DVE; self-loop contributions enter
        via PE transpose of the core's own fp32 rows; all matmuls accumulate
        (start=False) so the Tile scheduler may reorder them freely.
      * per supertile: ACT copies PSUM->SBUF, PE applies W (fp32), DVE scales
        by dis[row], DMA out.
  - Col-buckets of 25000 rows keep gather indices within int16 range.
  - One shared chunk schedule for all 8 cores (SPMD: one NEFF); per-core edge
    data is packed into the schedule with padding (idx=0 lanes get S=0 via
    slot id -1).
"""

import ml_dtypes
import numpy as np

import concourse.mybir as mybir
import concourse.tile as tile
from concourse import bacc
from concourse.bass_utils import run_bass_kernel_spmd
from concourse.masks import make_identity

N_NODES = 100000
N_EDGES = 1600000
D = 128
P = 128
NCORES = 8
RPC = N_NODES // NCORES            # rows per core = 12500
SLOTS = 512                        # slots per supertile (one PSUM bank, f32)
NST = (RPC + SLOTS - 1) // SLOTS   # 25 supertiles (last has 212 slots)
NBUCK = 4
BUCK = 25000                       # bucket size (int16-safe gather indices)
WWIN = 128                         # selection-matrix window width
GMAX = 8                           # max chunks per dma_gather call (1024 idx:
                                   # hard SWDGE ucode limit, 2048 crashes even
                                   # with a 32KB descriptor ring)
NQUEUES = 4                        # SWDGE queues; queue = bucket
GBUFS = 20                         # in-flight gather call tiles
F32 = mybir.dt.float32
BF16 = mybir.dt.bfloat16
I16 = mybir.dt.int16
BF = ml_dtypes.bfloat16

_compiled = {}


def _spread_bases(n_win, maxbase):
    """Evenly spread n_win window bases over [0, maxbase]."""
    if n_win <= 0:
        return []
    if n_win == 1:
        return [maxbase // 2 if maxbase > 0 else 0]
    return [i * maxbase // (n_win - 1) for i in range(n_win)]


def _chunk_layout(st, b, C):
    """Return (bases, widths) for C chunks of group (st, b)."""
    slots_st = min(SLOTS, RPC - st * SLOTS)
    maxbase = max(0, slots_st - WWIN)
    bases = _spread_bases(C, maxbase)
    widths = [WWIN] * C
    return bases, widths


def _assign(slots_arr, bases, widths):
    """Greedy interval assignment of edges (sorted by slot) to chunks.

    Returns list of per-chunk edge-index lists, or None if infeasible."""
    C = len(bases)
    E = len(slots_arr)
    cap = [[] for _ in range(C)]
    leftover = []
    ptr = 0
    for k in range(C):
        B = bases[k]
        end = B + widths[k]
        while ptr < E and slots_arr[ptr] < B:
            leftover.append(ptr)
            ptr += 1
        while ptr < E and slots_arr[ptr] < end and len(cap[k]) < P:
            cap[k].append(ptr)
            ptr += 1
    leftover.extend(range(ptr, E))
    for e in leftover:
        s = slots_arr[e]
        for k in range(C):
            if bases[k] <= s < bases[k] + widths[k] and len(cap[k]) < P:
                cap[k].append(e)
                break
        else:
            return None
    return cap


def _prepare(x, edge_index, W, nst_limit=NST):
    """Host-side preprocessing: degrees, xs tables, per-core packed metadata
    (gather indices + slot ids for on-chip S generation) and the shared
    bucket-major call schedule."""
    row = np.asarray(edge_index[0], dtype=np.int64)
    col = np.asarray(edge_index[1], dtype=np.int64)
    full_row = np.concatenate([row, np.arange(N_NODES, dtype=np.int64)])
    deg = np.bincount(full_row, minlength=N_NODES).astype(np.float64)
    dis = (1.0 / np.sqrt(deg)).astype(np.float32)
    xs32 = (x * dis[:, None]).astype(np.float32)
    xs16 = xs32.astype(BF)

    # only the original edges go through the gather path (self-loops are
    # handled by the transpose init)
    core = row // RPC
    lrow = (row - core * RPC).astype(np.int64)
    st_all = lrow // SLOTS
    slot_all = lrow % SLOTS
    buck_all = col // BUCK
    colrel_all = (col - buck_all * BUCK).astype(np.int64)

    order = np.lexsort((slot_all, buck_all, st_all, core))
    core_s = core[order]
    st_s = st_all[order]
    b_s = buck_all[order]
    slot_s = slot_all[order].astype(np.int64)
    colrel_s = colrel_all[order].astype(np.int64)

    key = ((core_s * NST) + st_s) * NBUCK + b_s
    bounds = np.searchsorted(key, np.arange(NCORES * NST * NBUCK + 1))

    def group(c, st, b):
        g = (c * NST + st) * NBUCK + b
        lo, hi = bounds[g], bounds[g + 1]
        return slot_s[lo:hi], colrel_s[lo:hi]

    # chunk counts: per-(st,b) max over cores of ceil(E/128), floored so the
    # evenly-spread windows cover every slot
    C = np.zeros((NST, NBUCK), dtype=np.int64)
    for st in range(nst_limit):
        slots_st = min(SLOTS, RPC - st * SLOTS)
        cover = max(1, -(-max(0, slots_st - WWIN) // WWIN) + 1)
        for b in range(NBUCK):
            mx = cover
            for c in range(NCORES):
                lo = bounds[(c * NST + st) * NBUCK + b]
                hi = bounds[(c * NST + st) * NBUCK + b + 1]
                mx = max(mx, -((lo - hi) // P))
            C[st, b] = mx

    assigns = {}
    for st in range(nst_limit):
        for b in range(NBUCK):
            while True:
                bases, widths = _chunk_layout(st, b, int(C[st, b]))
                ok = True
                for c in range(NCORES):
                    sl, _ = group(c, st, b)
                    a = _assign(sl, bases, widths)
                    if a is None:
                        ok = False
                        break
                    assigns[(c, st, b)] = a
                if ok:
                    break
                C[st, b] += 1
                if C[st, b] > 200:
                    raise RuntimeError(f"packing diverged at st={st} b={b}")

    # bucket-major global chunk order: (b, st, k); calls are consecutive runs
    # of up to GMAX chunks within one bucket (crossing supertile boundaries)
    schedule = []           # [st][b] -> (C, bases, widths)
    for st in range(nst_limit):
        per_b = []
        for b in range(NBUCK):
            bases, widths = _chunk_layout(st, b, int(C[st, b]))
            per_b.append((int(C[st, b]), bases, widths))
        schedule.append(per_b)

    gmap = {}               # (b, st, k) -> global chunk idx (bucket-major)
    gc = 0
    for b in range(NBUCK):
        for st in range(nst_limit):
            for k in range(int(C[st, b])):
                gmap[(b, st, k)] = gc
                gc += 1
    total_chunks = gc

    calls = []              # (b, g0, n): chunks g0..g0+n-1 (global idx)
    chunk_call = {}         # global chunk idx -> (call_idx, j)
    for b in range(NBUCK):
        nb = sum(int(C[st, b]) for st in range(nst_limit))
        g0 = gmap[(b, 0, 0)] if nb else 0
        off = 0
        while off < nb:
            n = min(GMAX, nb - off)
            for j in range(n):
                chunk_call[g0 + off + j] = (len(calls), j)
            calls.append((b, g0 + off, n))
            off += n

    idx_cols = total_chunks * (P // 16)

    slot_meta = np.full((NCORES, P, total_chunks), -1.0, dtype=BF)
    idx_meta = np.zeros((NCORES, P, idx_cols), dtype=np.int16)
    for c in range(NCORES):
        for st in range(nst_limit):
            for b in range(NBUCK):
                Cb, bases, widths = schedule[st][b]
                sl, cr = group(c, st, b)
                a = assigns[(c, st, b)]
                for k in range(Cb):
                    g = gmap[(b, st, k)]
                    edges = a[k]
                    ne = len(edges)
                    idx_flat = np.zeros(P, dtype=np.int16)
                    if ne:
                        e = np.asarray(edges, dtype=np.int64)
                        lanes = np.arange(ne)
                        slot_meta[c, lanes, g] = (sl[e] - bases[k]).astype(BF)
                        idx_flat[:ne] = cr[e].astype(np.int16)
                    wrapped = idx_flat.reshape(8, 16).T    # [16, 8]
                    idx_meta[c, :, g * 8:(g + 1) * 8] = np.tile(wrapped, (8, 1))

    ntiles = (RPC + P - 1) // P  # 98
    dis_meta = np.ones((NCORES, P, ntiles), dtype=np.float32)
    for c in range(NCORES):
        dd = dis[c * RPC:(c + 1) * RPC]
        pad = np.ones(ntiles * P, dtype=np.float32)
        pad[:RPC] = dd
        dis_meta[c] = pad.reshape(ntiles, P).T

    iota = np.tile(np.arange(WWIN, dtype=BF)[None, :], (P, 1))

    return (xs16, xs32, schedule, total_chunks, idx_cols, calls, chunk_call,
            gmap, slot_meta, idx_meta, dis_meta, iota)


def _build_program(schedule, total_chunks, idx_cols, calls, chunk_call, gmap):
    nst_limit = len(schedule)
    nc = bacc.Bacc("TRN2", target_bir_lowering=False, num_swdge_queues=NQUEUES)
    ntiles = (RPC + P - 1) // P

    xs_d = nc.dram_tensor("xs", [N_NODES, D], BF16, kind="ExternalInput")
    xself_d = nc.dram_tensor("xself", [RPC, D], F32, kind="ExternalInput")
    idx_d = nc.dram_tensor("idx", [P, idx_cols], I16, kind="ExternalInput")
    slot_d = nc.dram_tensor("slot", [P, total_chunks], BF16,
                            kind="ExternalInput")
    iota_d = nc.dram_tensor("iota", [P, WWIN], BF16, kind="ExternalInput")
    w_d = nc.dram_tensor("w", [D, D], F32, kind="ExternalInput")
    dis_d = nc.dram_tensor("dis", [P, ntiles], F32, kind="ExternalInput")
    out_d = nc.dram_tensor("out", [RPC, D], F32, kind="ExternalOutput")

    with tile.TileContext(nc) as tc:
        with tc.tile_pool(name="const", bufs=1) as const, \
             tc.tile_pool(name="g", bufs=GBUFS) as gp, \
             tc.tile_pool(name="sg", bufs=GBUFS) as sgp, \
             tc.tile_pool(name="xl", bufs=3) as xlp, \
             tc.tile_pool(name="misc", bufs=3) as misc, \
             tc.tile_pool(name="pacc", bufs=3, space="PSUM") as pacc, \
             tc.tile_pool(name="pout", bufs=2, space="PSUM") as pout:

            # load the index table in per-bucket slices (4 sub-slices each) on
            # the scalar (ACT) HWDGE ring, round-robin across buckets, so the
            # first gather of every bucket unblocks as early as possible
            idx_t = const.tile([P, idx_cols], I16, tag="idx")
            bstarts = [gmap[(b, 0, 0)] * 8 for b in range(NBUCK)] + [idx_cols]
            slices = []
            for b in range(NBUCK):
                lo, hi = bstarts[b], bstarts[b + 1]
                step = (hi - lo + 3) // 4
                slices.append([(s, min(s + step, hi))
                               for s in range(lo, hi, step)])
            for rnd in range(4):
                for b in range(NBUCK):
                    if rnd < len(slices[b]):
                        s, e = slices[b][rnd]
                        nc.scalar.dma_start(idx_t[:, s:e], idx_d[:, s:e])
                if rnd == 0:
                    slot_t = const.tile([P, total_chunks], BF16, tag="slot")
                    nc.scalar.dma_start(slot_t[:], slot_d[:, :])
                    iota_t = const.tile([P, WWIN], BF16, tag="iota")
                    nc.scalar.dma_start(iota_t[:], iota_d[:, :])
            w_t = const.tile([D, D], F32, tag="w")
            nc.sync.dma_start(w_t[:], w_d[:, :])
            dis_t = const.tile([P, ntiles], F32, tag="dis")
            nc.sync.dma_start(dis_t[:], dis_d[:, :])
            ident_t = const.tile([P, P], F32, tag="ident")
            make_identity(nc, ident_t[:])

            call_tiles = {}
            emitted = set()
            qrot = [0]

            def emit_call(cid):
                b, g0, n = calls[cid]
                gt = gp.tile([P, GMAX, P], BF16, tag="g")
                nrows = min(BUCK, N_NODES - b * BUCK)
                nc.gpsimd.dma_gather(
                    out_ap=gt[:, :n, :],
                    in_ap=xs_d[b * BUCK:b * BUCK + nrows, :],
                    idxs_ap=idx_t[:, g0 * 8:(g0 + n) * 8],
                    num_idxs=n * P,
                    num_idxs_reg=n * P,
                    elem_size=D,
                    queue_num=qrot[0] % NQUEUES,
                )
                qrot[0] += 1
                sgt = sgp.tile([P, GMAX, WWIN], BF16, tag="sg")
                nc.vector.tensor_tensor(
                    out=sgt[:, :n, :],
                    in0=iota_t[:, :].unsqueeze(1).broadcast_to([P, n, WWIN]),
                    in1=slot_t[:, g0:g0 + n].unsqueeze(2)
                        .broadcast_to([P, n, WWIN]),
                    op=mybir.AluOpType.is_equal,
                )
                call_tiles[cid] = (gt, sgt)
                emitted.add(cid)

            for st in range(nst_limit):
                accT = pacc.tile([P, SLOTS], F32, tag="acc")
                rows_st = min(SLOTS, RPC - st * SLOTS)
                nsub = (rows_st + P - 1) // P

                # zero the PSUM bank once; every matmul below accumulates, so
                # the scheduler may order them freely
                nc.vector.memset(accT[:], 0.0)

                # self-loop contributions via PE transpose of own rows
                for sub in range(nsub):
                    r0 = st * SLOTS + sub * P
                    rows = min(P, rows_st - sub * P)
                    xsel = xlp.tile([P, D], F32, tag="xl")
                    if rows < P:
                        nc.vector.memset(xsel[:], 0.0)
                    nc.sync.dma_start(xsel[:rows, :], xself_d[r0:r0 + rows, :])
                    nc.tensor.matmul(
                        out=accT[:, sub * P:(sub + 1) * P],
                        lhsT=xsel[:],
                        rhs=ident_t[:],
                        is_transpose=True,
                        start=False, stop=False,
                        skip_group_check=True,
                    )

                nchunks_st = sum(schedule[st][b][0] for b in range(NBUCK))
                done = 0
                for b in range(NBUCK):
                    Cb, bases, widths = schedule[st][b]
                    for k in range(Cb):
                        g = gmap[(b, st, k)]
                        cid, j = chunk_call[g]
                        if cid not in emitted:
                            emit_call(cid)
                        gt, sgt = call_tiles[cid]
                        done += 1
                        nc.tensor.matmul(
                            out=accT[:, bases[k]:bases[k] + widths[k]],
                            lhsT=gt[:, j, :],
                            rhs=sgt[:, j, :widths[k]],
                            start=False,
                            stop=(done == nchunks_st),
                            skip_group_check=True,
                        )

                accT_s = misc.tile([P, SLOTS], F32, tag="accs")
                nc.scalar.copy(out=accT_s[:], in_=accT[:])
                for sub in range(nsub):
                    rows = min(P, rows_st - sub * P)
                    op_t = pout.tile([P, D], F32, tag="op")
                    nc.tensor.matmul(
                        out=op_t[:],
                        lhsT=accT_s[:, sub * P:(sub + 1) * P],
                        rhs=w_t[:],
                        start=True, stop=True,
                    )
                    os_t = misc.tile([P, D], F32, tag="os")
                    nc.vector.tensor_scalar(
                        out=os_t[:],
                        in0=op_t[:],
                        scalar1=dis_t[:, st * 4 + sub:st * 4 + sub + 1],
                        scalar2=None,
                        op0=mybir.AluOpType.mult,
                    )
                    r0 = st * SLOTS + sub * P
                    nc.sync.dma_start(out_d[r0:r0 + rows, :], os_t[:rows, :])

    nc.compile()
    return nc


def kernel(x, edge_index, W, trace=False):
    import sys
    import time as _time
    x = np.ascontiguousarray(np.asarray(x, dtype=np.float32))
    edge_index = np.asarray(edge_index)
    W = np.ascontiguousarray(np.asarray(W, dtype=np.float32))

    t0 = _time.time()
    (xs16, xs32, schedule, total_chunks, idx_cols, calls, chunk_call, gmap,
     slot_meta, idx_meta, dis_meta, iota) = _prepare(x, edge_index, W)
    print(f"[kernel] prepare {_time.time()-t0:.1f}s, total_chunks={total_chunks}"
          f", calls={len(calls)}", file=sys.stderr)

    key = tuple(
        (schedule[st][b][0],) + tuple(schedule[st][b][1])
        for st in range(len(schedule)) for b in range(NBUCK)
    )
    if key not in _compiled:
        _compiled.clear()
        t0 = _time.time()
        _compiled[key] = _build_program(schedule, total_chunks, idx_cols,
                                        calls, chunk_call, gmap)
        print(f"[kernel] build+schedule {_time.time()-t0:.1f}s", file=sys.stderr)
    nc = _compiled[key]

    in_maps = []
    for c in range(NCORES):
        in_maps.append({
            "xs": xs16,
            "xself": np.ascontiguousarray(xs32[c * RPC:(c + 1) * RPC]),
            "idx": np.ascontiguousarray(idx_meta[c]),
            "slot": np.ascontiguousarray(slot_meta[c]),
            "iota": iota,
            "w": W,
            "dis": np.ascontiguousarray(dis_meta[c]),
        })

    res = run_bass_kernel_spmd(nc, in_maps, core_ids=list(range(NCORES)),
                               trace=trace)
    out = np.concatenate([res.results[c]["out"] for c in range(NCORES)], axis=0)
    kernel._last_results = res
    return out
